# revision 11
# baseline (speedup 1.0000x reference)
"""Trainium2 Bass kernel for nn_AttLayer (B=32, S=1024, D=1024, 8 NeuronCores).

Computation (per reference):
    qkv    = text @ W.T + b                      [B, S, D]
    scores = (qkv @ qkv^T per sample) / sqrt(D)  [B, S, S]
    attn   = softmax(scores, axis=0)             (softmax over the BATCH dim)
    out    = attn @ qkv                          [B, S, D]

Strategy: data-parallel over batch (4 samples per core). The batch softmax
only couples cores through T[q,k] = sum_b exp(scores[b,q,k]), a [S,S] f32
AllReduce (4 MB) fired immediately after the last sample's scores and
overlapped with the deferred qkv-transposes of the last two samples.
No max subtraction is needed: scores <= ~40 so exp stays inside f32 range.

Key implementation points:
 - the host passes text and W already transposed (layout prep during
   sharding), so no on-chip input transposes are needed; TensorE contracts
   over the partition dim, which wants text^T/W^T layouts.
 - matmuls run in float32r (TF32-like precision, ~1 cyc/row at N=512) for
   qkv/scores; bf16 for the attn@qkv output matmul. f32r operands must be
   produced by a rounding compute op, done in the SBUF load copies / PSUM
   evacuation copies.
 - scores/attn are bitwise symmetric per sample, so attn rows indexed by k
   serve directly as the transposed stationary operand of the final matmul.
 - E = exp(scores/32) (bf16) and qkv (bf16) spill to DRAM between the two
   phases; P = sum_local_b E accumulates in SBUF (4 chunk tiles for clean
   DMA dependencies) and is AllReduced via DRAM bounce buffers.
"""
import sys

sys.path.insert(0, "/opt/trn_rl_repo")

import numpy as np

import concourse.bacc as bacc
import concourse.mybir as mybir
import concourse.tile as tile
from concourse import masks
from concourse.bass_utils import run_bass_kernel_spmd

F32 = mybir.dt.float32
F32R = mybir.dt.float32r
BF16 = mybir.dt.bfloat16
EXP = mybir.ActivationFunctionType.Exp
COPY = mybir.ActivationFunctionType.Copy
IDENT = mybir.ActivationFunctionType.Identity

N_CORES = 8
B, S, D = 32, 1024, 1024
BL = B // N_CORES          # 4 local samples per core
NT = S // 128              # 8 partition tiles
NCH = 4                    # P accumulator chunks (2 q-tiles each)
SCALE = 1.0 / float(np.sqrt(D))

_nc_cache = {}


def _build():
    nc = bacc.Bacc("TRN2", target_bir_lowering=False, debug=False,
                   num_devices=N_CORES)
    # NOTE: host passes text as text^T per sample [BL, D, S] and W as W^T
    textT_d = nc.dram_tensor("text", [BL, D, S], F32, kind="ExternalInput")
    WT_d = nc.dram_tensor("W", [D, D], F32, kind="ExternalInput")
    bias = nc.dram_tensor("b", [D], F32, kind="ExternalInput")
    out = nc.dram_tensor("out", [BL, S, D], F32, kind="ExternalOutput")

    with tile.TileContext(nc) as tc:
        with (
            tc.tile_pool(name="outer", bufs=1) as outer,
            tc.tile_pool(name="dram", bufs=1, space="DRAM") as dram,
        ):
            # persistent DRAM spills / bounce buffers
            e_sp = dram.tile([BL, NT, 128, S], BF16)    # exp(scores), rows by q-tile
            q_sp = dram.tile([BL, NT, 128, D], BF16)    # qkv natural, rows by s-tile
            p_bnc = dram.tile([128, NT * S], BF16)
            t_bnc = dram.tile([128, NT * S], BF16, addr_space="Shared")

            # P accumulator in 4 chunk tiles so each chunk's DMA to the bounce
            # buffer depends only on its own writes
            Pc = [outer.tile([128, 2, S], F32, name=f"P{c}") for c in range(NCH)]
            ident = outer.tile([128, 128], F32)
            masks.make_identity(nc, ident[:])
            ident_b = outer.tile([128, 128], BF16)
            masks.make_identity(nc, ident_b[:])
            b_sb = outer.tile([128, NT], F32)
            nc.sync.dma_start(b_sb[:], bias.ap().rearrange("(t p) -> p t", p=128))

            # ---------------- phase 1 ----------------
            with (
                tc.tile_pool(name="ph1", bufs=1) as ph1,
                tc.tile_pool(name="ph1s", bufs=2) as ph1s,
                tc.tile_pool(name="ph1ps", bufs=1, space="PSUM") as pps,
            ):
                def copy_act(dst_ap, src_ap, bcol=None):
                    if bcol is None:
                        nc.scalar.activation(dst_ap, src_ap, COPY)
                    else:
                        nc.scalar.activation(dst_ap, src_ap, IDENT, bias=bcol)

                # W^T in f32r (DMA slab + ScalarE rounding copy)
                WT = ph1.tile([128, NT, D], F32R, tag="WT")
                for j in range(NT):
                    wslab = ph1s.tile([128, D], F32, tag="slab")
                    nc.sync.dma_start(wslab[:], WT_d.ap()[j * 128:(j + 1) * 128, :])
                    copy_act(WT[:, j, :], wslab[:])

                textT = ph1.tile([128, NT, S], F32R, tag="textT")
                # two qkvT buffers so the last two samples' qkv-transposes can
                # be deferred to overlap the AllReduce
                qkvTs = [ph1.tile([128, NT, S], F32R, name=f"qkvT{i}")
                         for i in range(2)]

                def qkv_transposes(b, qkvT):
                    # qkv natural (bf16) via PE transposes, spilled to DRAM
                    for st in range(NT):
                        qstage = ph1s.tile([128, D], BF16, tag="qstage")
                        for d4 in range(0, NT, 4):
                            pt = pps.tile([128, 512], F32, tag="tr", bufs=2)
                            for jj in range(4):
                                dt = d4 + jj
                                nc.tensor.transpose(
                                    pt[:, jj * 128:(jj + 1) * 128],
                                    qkvT[:, dt, st * 128:(st + 1) * 128].bitcast(F32),
                                    ident[:],
                                )
                            nc.scalar.activation(
                                qstage[:, d4 * 128:(d4 + 4) * 128], pt[:], COPY)
                        nc.sync.dma_start(q_sp[b, st], qstage[:])

                for b in range(BL):
                    qkvT = qkvTs[b % 2]
                    # text_b^T: DMA slab + DVE rounding copy to f32r
                    for j in range(NT):
                        tslab = ph1s.tile([128, S], F32, tag="slab")
                        nc.sync.dma_start(tslab[:],
                                          textT_d.ap()[b, j * 128:(j + 1) * 128, :])
                        nc.vector.tensor_copy(textT[:, j, :], tslab[:])

                    # qkvT[d, s] = sum_d' W[d, d'] * text[s, d']  (+ b[d])
                    for dt in range(NT):
                        pq = [pps.tile([128, 512], F32, tag="mm", bufs=6,
                                       name=f"pq{sc}") for sc in range(2)]
                        for kt in range(NT):
                            for sc in range(2):
                                nc.tensor.matmul(
                                    pq[sc][:],
                                    WT[:, kt, dt * 128:(dt + 1) * 128],
                                    textT[:, kt, sc * 512:(sc + 1) * 512],
                                    start=(kt == 0),
                                    stop=(kt == NT - 1),
                                )
                        for sc in range(2):
                            copy_act(qkvT[:, dt, sc * 512:(sc + 1) * 512],
                                     pq[sc][:], bcol=b_sb[:, dt:dt + 1])

                    # scores (upper triangle at 512-chunk granularity) + exp;
                    # lower-left E blocks reconstructed by transposing upper
                    # blocks (scores is symmetric). Then P accumulation + spill.
                    ef = ph1s.tile([128, NT, S], BF16, tag="ef", bufs=1)

                    def score_row(qt, kcs):
                        psc = [pps.tile([128, 512], F32, tag="mm", bufs=6,
                                        name=f"psc{kc}") for kc in kcs]
                        for dt in range(NT):
                            for j, kc in enumerate(kcs):
                                nc.tensor.matmul(
                                    psc[j][:],
                                    qkvT[:, dt, qt * 128:(qt + 1) * 128],
                                    qkvT[:, dt, kc * 512:(kc + 1) * 512],
                                    start=(dt == 0),
                                    stop=(dt == NT - 1),
                                )
                        for j, kc in enumerate(kcs):
                            nc.scalar.activation(
                                ef[:, qt, kc * 512:(kc + 1) * 512], psc[j][:],
                                EXP, scale=float(SCALE))

                    def finish_row(qt):
                        pc, h = Pc[qt // 2], qt % 2
                        if b == 0:
                            nc.vector.tensor_copy(pc[:, h, :], ef[:, qt, :])
                        else:
                            nc.vector.tensor_add(pc[:, h, :], pc[:, h, :],
                                                 ef[:, qt, :])
                        nc.sync.dma_start(e_sp[b, qt], ef[:, qt, :])
                        if b == BL - 1 and qt % 2 == 1:
                            c = qt // 2
                            pcast = ph1s.tile([128, 2 * S], BF16, tag="pcast")
                            nc.scalar.activation(
                                pcast[:], pc[:].rearrange("p t s -> p (t s)"),
                                COPY)
                            nc.sync.dma_start(
                                p_bnc[:, c * 2 * S:(c + 1) * 2 * S], pcast[:])

                    for qt in range(4):
                        score_row(qt, (0, 1))
                        finish_row(qt)
                    # E[ct, rt*128:(rt+1)*128] = E[rt, ct*128:(ct+1)*128]^T
                    for ct in range(4, NT):
                        pt = pps.tile([128, 512], BF16, tag="tr", bufs=2)
                        for rt in range(4):
                            nc.tensor.transpose(
                                pt[:, rt * 128:(rt + 1) * 128],
                                ef[:, rt, ct * 128:(ct + 1) * 128],
                                ident_b[:],
                            )
                        nc.vector.tensor_copy(ef[:, ct, 0:512], pt[:])
                    for qt in range(4, NT):
                        score_row(qt, (1,))
                        finish_row(qt)

                    # qkv transposes inline for the first samples; deferred
                    # for the last two so they overlap the AllReduce
                    if b < BL - 2:
                        qkv_transposes(b, qkvT)

                nc.gpsimd.collective_compute(
                    "AllReduce",
                    mybir.AluOpType.add,
                    replica_groups=[list(range(N_CORES))],
                    ins=[p_bnc[:].opt()],
                    outs=[t_bnc[:].opt()],
                )
                qkv_transposes(BL - 2, qkvTs[(BL - 2) % 2])
                qkv_transposes(BL - 1, qkvTs[(BL - 1) % 2])

            # ---------------- phase 2 ----------------
            with (
                tc.tile_pool(name="ph2", bufs=1) as ph2,
                tc.tile_pool(name="ph2s", bufs=2) as ph2s,
                tc.tile_pool(name="ph2ps", bufs=1, space="PSUM") as pps2,
            ):
                R = ph2.tile([128, NT, S], F32, tag="R")
                # prefetch the first two samples' streams BEFORE the T loads:
                # the sync engine runs its stream in order, and the T loads
                # block on the collective, so these must come first to overlap
                # the AllReduce
                e_bs, qkv_bs = {}, {}
                for b in range(1):
                    e_bs[b] = ph2s.tile([128, NT, S], BF16, tag="e_b",
                                        name=f"e_b{b}")
                    qkv_bs[b] = ph2s.tile([128, NT, D], BF16, tag="qkv_b",
                                          name=f"qkv_b{b}")
                    nc.sync.dma_start(e_bs[b][:],
                                      e_sp[b].rearrange("t p s -> p t s"))
                    nc.sync.dma_start(qkv_bs[b][:],
                                      q_sp[b].rearrange("t p s -> p t s"))
                # fine-grained recip so the first attn rows are ready ASAP
                for qt in range(NT):
                    tstage_b = ph2s.tile([128, S], BF16, tag="tstage_b")
                    tstage = ph2s.tile([128, S], F32, tag="tstage")
                    nc.sync.dma_start(tstage_b[:], t_bnc[:, qt * S:(qt + 1) * S])
                    for hh in range(2):
                        nc.scalar.activation(
                            tstage[:, hh * 512:(hh + 1) * 512],
                            tstage_b[:, hh * 512:(hh + 1) * 512], COPY)
                        nc.vector.reciprocal_approx_fast(
                            R[:, qt, hh * 512:(hh + 1) * 512],
                            tstage[:, hh * 512:(hh + 1) * 512])

                for b in range(BL):
                    if b < 1:
                        e_b, qkv_b = e_bs[b], qkv_bs[b]
                    else:
                        e_b = ph2s.tile([128, NT, S], BF16, tag="e_b",
                                        name=f"e_b{b}")
                        qkv_b = ph2s.tile([128, NT, D], BF16, tag="qkv_b",
                                          name=f"qkv_b{b}")
                        nc.sync.dma_start(e_b[:],
                                          e_sp[b].rearrange("t p s -> p t s"))
                        nc.sync.dma_start(qkv_b[:],
                                          q_sp[b].rearrange("t p s -> p t s"))
                    attn_b = ph2s.tile([128, NT, S], BF16, tag="attn_b")
                    for qt in range(NT):
                        if b == 0:
                            for hh in range(2):
                                nc.vector.tensor_mul(
                                    attn_b[:, qt, hh * 512:(hh + 1) * 512],
                                    e_b[:, qt, hh * 512:(hh + 1) * 512],
                                    R[:, qt, hh * 512:(hh + 1) * 512])
                        else:
                            nc.vector.tensor_mul(attn_b[:, qt, :],
                                                 e_b[:, qt, :], R[:, qt, :])
                    # out[q, d] = sum_k attn[q, k] qkv[k, d]; attn is symmetric,
                    # so rows of attn_b indexed by k give lhsT[k, q] directly.
                    for qt in range(NT):
                        ostage = ph2s.tile([128, D], F32, tag="ostage")
                        po = [pps2.tile([128, 512], F32, tag="mmo", bufs=6,
                                        name=f"po{dc}") for dc in range(2)]
                        for kt in range(NT):
                            for dc in range(2):
                                nc.tensor.matmul(
                                    po[dc][:],
                                    attn_b[:, kt, qt * 128:(qt + 1) * 128],
                                    qkv_b[:, kt, dc * 512:(dc + 1) * 512],
                                    start=(kt == 0),
                                    stop=(kt == NT - 1),
                                )
                        for dc in range(2):
                            nc.scalar.activation(
                                ostage[:, dc * 512:(dc + 1) * 512], po[dc][:],
                                COPY)
                        nc.sync.dma_start(
                            out.ap()[b, qt * 128:(qt + 1) * 128, :], ostage[:])

    nc.compile()
    return nc


def _get_nc():
    if "nc" not in _nc_cache:
        _nc_cache["nc"] = _build()
    return _nc_cache["nc"]


def _run(text, W, b, trace=False):
    text = np.asarray(text, dtype=np.float32)
    W = np.asarray(W, dtype=np.float32)
    b = np.ascontiguousarray(b, dtype=np.float32)
    # host-side layout prep: per-sample transposed text, transposed W
    WT = np.ascontiguousarray(W.T)
    shards = np.split(text, N_CORES, axis=0)
    in_maps = [
        {"text": np.ascontiguousarray(shards[i].transpose(0, 2, 1)),
         "W": WT, "b": b}
        for i in range(N_CORES)
    ]
    nc = _get_nc()
    res = run_bass_kernel_spmd(nc, in_maps, core_ids=list(range(N_CORES)),
                               trace=trace)
    full = np.concatenate([res.results[i]["out"] for i in range(N_CORES)],
                          axis=0)
    return full, res


def kernel(text, W, b):
    full, _ = _run(text, W, b, trace=False)
    return full
